# revision 39
# baseline (speedup 1.0000x reference)
"""Trainium2 Bass kernel for hyperbolic linear-attention transformer layer.

Data-parallel over nodes (N=32768) across 8 NeuronCores.

Math (per rep):
  Phase A (source nodes, node-major tiles of 128):
    k = Wk x_pad  (PE, fp32r) ; z = relu(k) (ACT)
    y = z^2 with per-head sums sy (DVE tensor_tensor_reduce)
    sy2 = sum(y^2) per head (ACT Square accum / DVE ttr)
    phi_k = y * sqrt(sy/sy2)   (in-place)
    B += x_pad^T phi_k         (PE, PSUM accumulation over node tiles)
      - x_pad has a trailing 1-column, so row 257 of B is sum_n(phi_k) "sumk"
  AllReduce(B) over the 8 cores  (2.1 MB, Shared output).
  Mid: G[h] = B_h^T WG_h where WG_h = Wv_pad_h fw_h^T (host-precomputed);
    this folds ktv = B^T Wv and the final projection fw into one matrix.
    sumk columns are DMA-staged into the stats lhsT (zt2).
  Phase B (query nodes, feature-major supertiles of 512):
    q = Wq x_pad (PE) ; z = relu(q) ; y = z^2 ; y2 = y^2
    A,C sums via one matmul per 128-feat chunk (lhsT = [ind | sumk]),
    Bsum via matmul vs y2.  fac = sqrt(A/Bsum); s = fac/(C*fac + eps).
    phi' = y * s (broadcast via K=8 matmul)
    outT = sum_h G_h^T phi'_h + W2 x_s  (W2 = fw @ (vmap Wv) folded on host,
      bias row included)
    PE-transpose outT -> node-major, Lorentz lift, DMA out.

All matmuls are fp32r with moving dim >= 256 (full PE rate).
"""

import os
import numpy as np
import concourse.bass as bass
import concourse.tile as tile
from concourse import bacc, mybir
from concourse.bass_utils import run_bass_kernel_spmd

F32 = mybir.dt.float32
F32R = mybir.dt.float32r
BF16 = mybir.dt.bfloat16
AF = mybir.ActivationFunctionType
ALU = mybir.AluOpType

NCORES = 8
N = 32768
NCHUNK = N // NCORES          # 4096 nodes per core
H = 8
D = 256
HD = H * D                    # 2048
KC = 3                        # contraction chunks: 384 = 3*128 (258 used)
EPS = 1e-6
NST = 512                     # phase-B supertile node count

_CACHE = {}


def _build(reps=1):
    if reps in _CACHE:
        return _CACHE[reps]
    nc = bacc.Bacc("TRN2", target_bir_lowering=False, debug=False,
                   num_devices=NCORES)

    xqT = nc.dram_tensor("xqT", [KC, 128, NCHUNK], F32R, kind="ExternalInput").ap()
    xsT = nc.dram_tensor("xsT", [KC, 128, NCHUNK], F32R, kind="ExternalInput").ap()
    xnm = nc.dram_tensor("xnm", [NCHUNK, KC * 128], F32R, kind="ExternalInput").ap()
    wq = nc.dram_tensor("wq", [KC, 128, HD], F32R, kind="ExternalInput").ap()
    wk = nc.dram_tensor("wk", [KC, 128, HD], F32R, kind="ExternalInput").ap()
    w2 = nc.dram_tensor("w2", [KC, 128, D], F32R, kind="ExternalInput").ap()
    wg = nc.dram_tensor("wg", [KC, 128, H, D], F32R, kind="ExternalInput").ap()
    zt2 = nc.dram_tensor("zt2", [128, 16, 8], F32R, kind="ExternalInput").ap()
    ind = nc.dram_tensor("ind", [128, 8, 8], F32R, kind="ExternalInput").ap()
    ind2 = nc.dram_tensor("ind2", [8, 8, 128], F32R, kind="ExternalInput").ap()
    ident = nc.dram_tensor("ident", [128, 128], F32R, kind="ExternalInput").ap()
    cons = nc.dram_tensor("cons", [8, 1], F32, kind="ExternalInput").ap()
    out = nc.dram_tensor("out", [NCHUNK, 257], F32, kind="ExternalOutput").ap()

    with tile.TileContext(nc) as tc:
        _body(nc, tc, reps, xqT, xsT, xnm, wq, wk, w2, wg, zt2, ind, ind2,
              ident, cons, out)
    nc.compile()
    _CACHE[reps] = nc
    return nc


def _body(nc, tc, reps, xqT, xsT, xnm, wq, wk, w2, wg, zt2, ind, ind2,
          ident, cons, out):
    import contextlib
    stack = contextlib.ExitStack()
    with stack:
        cpool = stack.enter_context(tc.tile_pool(name="const", bufs=1))
        dpool = stack.enter_context(tc.tile_pool(name="dram", bufs=1, space="DRAM"))

        ind_sb = cpool.tile([128, 8, 8], F32R)
        nc.sync.dma_start(ind_sb[:], ind[:])
        ind2_sb = cpool.tile([8, 8, 128], F32R)
        nc.sync.dma_start(ind2_sb[:], ind2[:])
        ident_sb = cpool.tile([128, 128], F32R)
        nc.sync.dma_start(ident_sb[:], ident[:])
        eps_sb = cpool.tile([8, 1], F32)
        nc.sync.dma_start(eps_sb[:], cons[:])
        zt_sb = cpool.tile([128, 16, 8], F32R)
        nc.sync.dma_start(zt_sb[:], zt2[:])

        for rep in range(reps):
            ar_in = [dpool.tile([258, 1024], BF16, tag=f"ari{rep}g{g}",
                                name=f"ari{rep}g{g}") for g in range(2)]
            ar_out = [dpool.tile([258, 1024], BF16,
                                 tag=f"aro{rep}g{g}", name=f"aro{rep}g{g}")
                      for g in range(2)]
            if not os.environ.get("KT_SKIP_A"):
                _phase_a(nc, tc, xsT, xnm, wk, ar_in, ar_out)
            if not os.environ.get("KT_SKIP_B"):
                _phase_b(nc, tc, xqT, xsT, wq, w2, wg, zt_sb, ind_sb, ind2_sb,
                         ident_sb, eps_sb, ar_out, out)
            else:
                with tc.tile_pool(name="oBtmp", bufs=1) as ob:
                    o_sb = ob.tile([128, 257], F32)
                    nc.sync.dma_start(o_sb[:], ar_out[0][0:128, 0:257])
                    for t0_ in range(NCHUNK // 128):
                        nc.sync.dma_start(out[t0_ * 128:(t0_ + 1) * 128, :], o_sb[:])


def _phase_a(nc, tc, xsT, xnm, wk, ar_in, ar_out):
    import contextlib
    with contextlib.ExitStack() as st:
        wpool = st.enter_context(tc.tile_pool(name="wA", bufs=1))
        xp = st.enter_context(tc.tile_pool(name="xA", bufs=3))
        xnp_ = st.enter_context(tc.tile_pool(name="xnA", bufs=4))
        yp = st.enter_context(tc.tile_pool(name="yA", bufs=3))
        zap = st.enter_context(tc.tile_pool(name="zA", bufs=2))
        scp = st.enter_context(tc.tile_pool(name="scA", bufs=2))
        stp = st.enter_context(tc.tile_pool(name="stA", bufs=4))
        drp = st.enter_context(tc.tile_pool(name="drA", bufs=2))
        pk = st.enter_context(tc.tile_pool(name="psAk", bufs=2, space="PSUM"))
        pb = st.enter_context(tc.tile_pool(name="psAb", bufs=1, space="PSUM"))

        wk_sb = wpool.tile([128, KC, HD], F32R)
        nc.sync.dma_start(wk_sb[:], wk.rearrange("c p n -> p c n"))

        ntiles = NCHUNK // 128
        for g in range(2):
            gofs = g * 1024
            b_ps0 = pb.tile([128, 1024], F32, tag="bps0")
            b_ps1 = pb.tile([128, 1024], F32, tag="bps1")
            b_ps2 = pb.tile([2, 1024], F32, tag="bps2")
            b_tiles = (b_ps0, b_ps1, b_ps2)
            prev = None
            onex = bool(os.environ.get("KT_ONEX"))
            for t in range(ntiles):
                tsrc = 0 if onex else t
                xs_sb = xp.tile([128, KC, 128], F32R, tag="xs")
                nc.sync.dma_start(
                    xs_sb[:],
                    xsT[:, :, tsrc * 128:(tsrc + 1) * 128].rearrange(
                        "c p n -> p c n"))
                xn_sb = xnp_.tile([128, KC, 128], F32R, tag="xn")
                nc.sync.dma_start(
                    xn_sb[:],
                    xnm[tsrc * 128:(tsrc + 1) * 128, :].rearrange(
                        "n (c f) -> n c f", c=KC))

                y = yp.tile([128, 1024], F32R, tag="y")
                yf = y.bitcast(F32)
                z = zap.tile([128, 1024], F32, tag="zA")
                sy = stp.tile([128, 4], F32, tag="sy")
                sy2 = stp.tile([128, 4], F32, tag="sy2")
                for blk in range(2):
                    kp = pk.tile([128, 512], F32, tag="kp")
                    for c in range(KC):
                        nc.tensor.matmul(
                            kp[:], lhsT=xs_sb[:, c],
                            rhs=wk_sb[:, c, gofs + blk * 512: gofs + blk * 512 + 512],
                            start=(c == 0), stop=(c == KC - 1))
                    nc.vector.tensor_scalar_max(
                        z[:, blk * 512:(blk + 1) * 512], kp[:], 0.0)
                if os.environ.get("KT_NOSTATS"):
                    for hh in range(4):
                        sl = slice(hh * 256, hh * 256 + 256)
                        nc.scalar.activation(y[:, sl], z[:, sl], AF.Square)
                else:
                    # y = z^2 with per-head accumulated sums sy (ACT)
                    for hh in range(4):
                        sl = slice(hh * 256, hh * 256 + 256)
                        nc.scalar.activation(y[:, sl], z[:, sl], AF.Square,
                                             accum_out=sy[:, hh:hh + 1])
                    # sy2 = sum(y^2) per head (ACT Square with accum)
                    for hh in range(4):
                        sl = slice(hh * 256, hh * 256 + 256)
                        scr = scp.tile([128, 256], F32, tag="scr")
                        nc.scalar.activation(scr[:], yf[:, sl], AF.Square,
                                             accum_out=sy2[:, hh:hh + 1])
                    rec = stp.tile([128, 4], F32, tag="rec")
                    nc.vector.reciprocal(rec[:], sy2[:])
                    rat = stp.tile([128, 4], F32, tag="rat")
                    nc.vector.tensor_mul(rat[:], sy[:], rec[:])
                    fac = stp.tile([128, 4], F32, tag="fac")
                    nc.scalar.activation(fac[:], rat[:], AF.Sqrt)
                    # phi = y * fac, in place (2 on DVE, 2 on ACT)
                    for hh in range(4):
                        sl = slice(hh * 256, hh * 256 + 256)
                        if hh < 2:
                            nc.vector.tensor_scalar_mul(y[:, sl], yf[:, sl],
                                                        fac[:, hh:hh + 1])
                        else:
                            nc.scalar.activation(y[:, sl], yf[:, sl], AF.Copy,
                                                 scale=fac[:, hh:hh + 1])

                # B accumulation for the PREVIOUS tile (software pipeline:
                # keeps PE busy with tile t's projection while tile t-1's
                # phi chain drains on DVE/ACT)
                if os.environ.get("KT_NO_B"):
                    if t == 0:
                        _b_accum(nc, b_tiles, xn_sb, y, 0, 1)
                elif t > 0:
                    _b_accum(nc, b_tiles, prev[0], prev[1], t - 1, ntiles)
                prev = (xn_sb, y)
            if not os.environ.get("KT_NO_B"):
                _b_accum(nc, b_tiles, prev[0], prev[1], ntiles - 1, ntiles)

            d0 = drp.tile([128, 1024], BF16, tag="d0")
            nc.vector.tensor_copy(d0[:], b_ps0[:])
            nc.sync.dma_start(ar_in[g][0:128, :], d0[:])
            d1 = drp.tile([128, 1024], BF16, tag="d1")
            nc.scalar.copy(d1[:], b_ps1[:])
            nc.sync.dma_start(ar_in[g][128:256, :], d1[:])
            d2 = drp.tile([2, 1024], BF16, tag="d2")
            nc.vector.tensor_copy(d2[:], b_ps2[:])
            nc.sync.dma_start(ar_in[g][256:258, :], d2[:])
            # per-group AllReduce: group 0's AR overlaps group 1's compute
            if os.environ.get("KT_LOCAL_AR"):
                nc.sync.dma_start(ar_out[g][:], ar_in[g][:])
            else:
                nc.gpsimd.collective_compute(
                    "AllReduce", ALU.add,
                    replica_groups=[list(range(NCORES))],
                    ins=[ar_in[g].opt()], outs=[ar_out[g].opt()])


def _b_accum(nc, b_tiles, xn_sb, y, t, ntiles):
    b_ps0, b_ps1, b_ps2 = b_tiles
    for blk in range(2):
        ms = slice(blk * 512, blk * 512 + 512)
        nc.tensor.matmul(b_ps0[:, ms], lhsT=xn_sb[:, 0], rhs=y[:, ms],
                         start=(t == 0), stop=(t == ntiles - 1))
        nc.tensor.matmul(b_ps1[:, ms], lhsT=xn_sb[:, 1], rhs=y[:, ms],
                         start=(t == 0), stop=(t == ntiles - 1))
        nc.tensor.matmul(b_ps2[:, ms], lhsT=xn_sb[:, 2, 0:2], rhs=y[:, ms],
                         start=(t == 0), stop=(t == ntiles - 1))


def _phase_b(nc, tc, xqT, xsT, wq, w2, wg, zt_sb, ind_sb, ind2_sb,
             ident_sb, eps_sb, ar_out, out):
    import contextlib
    with contextlib.ExitStack() as st:
        wpool = st.enter_context(tc.tile_pool(name="wB", bufs=1))
        mpool = st.enter_context(tc.tile_pool(name="midB", bufs=1))
        xp = st.enter_context(tc.tile_pool(name="xB", bufs=2))
        yp = st.enter_context(tc.tile_pool(name="yB", bufs=34))
        zbp = st.enter_context(tc.tile_pool(name="zbB", bufs=3))
        scp2 = st.enter_context(tc.tile_pool(name="sc2B", bufs=2))
        stp = st.enter_context(tc.tile_pool(name="stB", bufs=1))
        sbp = st.enter_context(tc.tile_pool(name="sbB", bufs=2))
        asb = st.enter_context(tc.tile_pool(name="aB", bufs=3))
        obp = st.enter_context(tc.tile_pool(name="oB", bufs=3))
        ps = st.enter_context(tc.tile_pool(name="psBs", bufs=2, space="PSUM"))
        pa = st.enter_context(tc.tile_pool(name="psBa", bufs=3, space="PSUM"))
        pst = st.enter_context(tc.tile_pool(name="psBt", bufs=2, space="PSUM"))

        wq_sb = wpool.tile([128, KC, HD], F32R)
        nc.sync.dma_start(wq_sb[:], wq.rearrange("c p n -> p c n"))
        w2_sb = wpool.tile([128, KC, D], F32R)
        nc.sync.dma_start(w2_sb[:], w2.rearrange("c p n -> p c n"))
        g_sb = wpool.tile([128, H, 2, D], F32R)

        def mid_g(g):
            # G = B^T WG for this half's heads; stage sumk cols into zt
            bf0 = mpool.tile([128, 1024], F32R, tag="bf0")
            nc.gpsimd.dma_start(bf0[:], ar_out[g][0:128, :])
            bf1 = mpool.tile([128, 1024], F32R, tag="bf1")
            nc.gpsimd.dma_start(bf1[:], ar_out[g][128:256, :])
            bf2 = mpool.tile([2, 1024], F32R, tag="bf2")
            nc.gpsimd.dma_start(bf2[:], ar_out[g][256:258, :])
            wg_sb = mpool.tile([128, KC, 4, D], F32R, tag="wgh")
            nc.sync.dma_start(
                wg_sb[:], wg[:, :, g * 4:(g + 1) * 4].rearrange(
                    "c p h n -> p c h n"))
            bfs = [bf0, bf1, bf2]
            for hl in range(4):
                hh = g * 4 + hl
                for mc in range(2):
                    msl = slice(hl * 256 + mc * 128, hl * 256 + mc * 128 + 128)
                    gp = pa.tile([128, NST], F32, tag="mm")
                    for fc in range(KC):
                        lhs = bfs[fc][:, msl] if fc < 2 else bfs[2][0:2, msl]
                        rhs = (wg_sb[:, fc, hl] if fc < 2
                               else wg_sb[0:2, fc, hl])
                        nc.tensor.matmul(gp[:, 0:D], lhsT=lhs, rhs=rhs,
                                         start=(fc == 0), stop=(fc == KC - 1))
                    if (hl * 2 + mc) % 2 == 0:
                        nc.vector.tensor_copy(g_sb[:, hh, mc], gp[:, 0:D])
                    else:
                        nc.scalar.copy(g_sb[:, hh, mc], gp[:, 0:D])
            for cl in range(8):
                c = g * 8 + cl
                hh = c // 2
                nc.gpsimd.dma_start(
                    zt_sb[:, c, hh:hh + 1],
                    ar_out[g][257:258, cl * 128:(cl + 1) * 128].rearrange(
                        "r (p o) -> (r p) o", o=1))

        state = {}

        def pre(stx):
            nofs = stx * NST
            xq_sb = xp.tile([128, KC, NST], F32R, tag="xq")
            nc.sync.dma_start(
                xq_sb[:], xqT[:, :, nofs:nofs + NST].rearrange("c p n -> p c n"))
            ys = []
            for c in range(16):
                qp = pa.tile([128, NST], F32, tag="mm")
                for kc in range(KC):
                    nc.tensor.matmul(
                        qp[:], lhsT=wq_sb[:, kc, c * 128:(c + 1) * 128],
                        rhs=xq_sb[:, kc], start=(kc == 0), stop=(kc == KC - 1))
                zb = zbp.tile([128, NST], F32, tag="zbB")
                if c % 4 == 3:
                    nc.scalar.activation(zb[:], qp[:], AF.Relu)
                else:
                    nc.vector.tensor_scalar_max(zb[:], qp[:], 0.0)
                y = yp.tile([128, NST], F32R, tag="yB")
                nc.scalar.activation(y[:], zb[:], AF.Square)
                ys.append(y)
            state[stx] = ys

        def post(stx):
            nofs = stx * NST
            ys = state.pop(stx)
            # C = sumk-weighted sums of y; query-side fp-normalization cancels
            # between numerator and denominator, so s = 1/(C + eps) directly
            sums_ps = ps.tile([8, NST], F32, tag="sums")
            for c in range(16):
                nc.tensor.matmul(sums_ps[:], lhsT=zt_sb[:, c], rhs=ys[c][:],
                                 start=(c == 0), stop=(c == 15))
            den = stp.tile([8, NST], F32, tag="denB")
            nc.vector.tensor_scalar_add(den[:], sums_ps[:], eps_sb[:])
            s_sb = stp.tile([8, NST], F32R, tag="sB")
            with nc.allow_low_precision(reason="f32r rounding for matmul rhs"):
                nc.vector.reciprocal(s_sb[:], den[:])

            # phi' = y * s (broadcast s across partitions via K=8 matmul)
            for hh in range(8):
                sb_ps = pst.tile([128, NST], F32, tag="sbtr")
                nc.tensor.matmul(sb_ps[:], lhsT=ind2_sb[:, hh], rhs=s_sb[:],
                                 start=True, stop=True)
                sbc = sbp.tile([128, NST], F32, tag="sbcs")
                if hh % 4 == 3:
                    nc.vector.tensor_copy(sbc[:], sb_ps[:])
                else:
                    nc.scalar.copy(sbc[:], sb_ps[:])
                for mc in range(2):
                    yo = ys[2 * hh + mc]
                    nc.vector.tensor_mul(yo[:], yo.bitcast(F32)[:], sbc[:])

            # outT accumulation: vss (W2 x_s) then numerator via G
            xs_sb = xp.tile([128, KC, NST], F32R, tag="xsB")
            nc.sync.dma_start(
                xs_sb[:], xsT[:, :, nofs:nofs + NST].rearrange("c p n -> p c n"))
            at_sbs = []
            for oc in range(2):
                osl = slice(oc * 128, oc * 128 + 128)
                at_ps = pa.tile([128, NST], F32, tag="mm")
                for fc in range(KC):
                    nc.tensor.matmul(at_ps[:], lhsT=w2_sb[:, fc, osl],
                                     rhs=xs_sb[:, fc],
                                     start=(fc == 0), stop=False)
                for hh in range(8):
                    for mc in range(2):
                        nc.tensor.matmul(
                            at_ps[:], lhsT=g_sb[:, hh, mc, osl],
                            rhs=ys[2 * hh + mc][:],
                            start=False, stop=(hh == 7 and mc == 1))
                at_sb = asb.tile([128, NST], F32R, tag="atB")
                if oc == 0:
                    nc.scalar.copy(at_sb[:], at_ps[:])
                else:
                    nc.vector.tensor_copy(at_sb[:], at_ps[:])
                at_sbs.append(at_sb)

            # transpose to node-major + Lorentz lift
            for sn in range(NST // 128):
                tr_ps = pst.tile([128, 2, 128], F32R, tag="sbtr")
                for oc in range(2):
                    nc.tensor.transpose(
                        tr_ps[:, oc],
                        at_sbs[oc][:, sn * 128:(sn + 1) * 128], ident_sb[:])
                trf = tr_ps.bitcast(F32).rearrange("p a b -> p (a b)")
                o_sb = obp.tile([128, 257], F32, tag="osb")
                nc.vector.tensor_copy(o_sb[:, 1:257], trf)
                scr2 = scp2.tile([128, 256], F32, tag="scr2")
                ssum = stp.tile([128, 1], F32, tag="ssum")
                nc.scalar.activation(scr2[:], trf, AF.Square, accum_out=ssum[:])
                nc.scalar.activation(o_sb[:, 0:1], ssum[:], AF.Sqrt, bias=1.0)
                nc.sync.dma_start(out[nofs + sn * 128: nofs + (sn + 1) * 128, :],
                                  o_sb[:])

        # software pipeline: pres are AR-independent, posts need G/zt (AR)
        nst = NCHUNK // NST
        mid_g(0)
        pre(0)
        pre(1)
        mid_g(1)
        post(0)
        for stx in range(2, nst):
            pre(stx)
            post(stx - 1)
        post(nst - 1)


def _prep_inputs(query_input, source_input, Wq_w, Wq_b, Wk_w, Wk_b, Wv_w, Wv_b,
                 norm_scale, v_map_w, v_map_b, final_w, final_b):
    def pad_xT(x):
        xt = np.zeros((KC * 128, N), np.float32)
        xt[0:257] = x.T
        xt[257] = 1.0
        return xt.reshape(KC, 128, N)

    def pad_w(w_flat, b_flat):
        wt = np.zeros((KC * 128, HD), np.float32)
        wt[0:257] = w_flat.T
        wt[257] = b_flat
        return wt.reshape(KC, 128, HD)

    xq = np.asarray(query_input, np.float32)
    xs = np.asarray(source_input, np.float32)
    xqT = pad_xT(xq)
    xsT = pad_xT(xs)
    xnm = np.zeros((N, KC * 128), np.float32)
    xnm[:, 0:257] = xs
    xnm[:, 257] = 1.0

    wq_h = pad_w(np.asarray(Wq_w).reshape(HD, 257), np.asarray(Wq_b).reshape(HD))
    wk_h = pad_w(np.asarray(Wk_w).reshape(HD, 257), np.asarray(Wk_b).reshape(HD))

    fw = np.asarray(final_w, np.float64)                     # [256, 2048]
    vm = np.asarray(v_map_w, np.float64)
    wv_flat = np.asarray(Wv_w, np.float64).reshape(HD, 257)  # [2048, 257]
    wv_b = np.asarray(Wv_b, np.float64).reshape(HD)

    # W2 = fw @ (vm @ Wv) : [256, 257]; bias = fw @ (vm @ Wv_b + v_map_b) + final_b
    wvm_flat = np.einsum('od,hdi->hoi', vm, np.asarray(Wv_w, np.float64)
                         ).reshape(HD, 257)
    bvm = (np.asarray(Wv_b, np.float64) @ vm.T
           + np.asarray(v_map_b, np.float64)[None, :]).reshape(HD)
    w2m = fw @ wvm_flat                                      # [256, 257]
    b2 = fw @ bvm + np.asarray(final_b, np.float64)          # [256]
    w2_h = np.zeros((KC * 128, D), np.float32)
    w2_h[0:257] = w2m.T.astype(np.float32)
    w2_h[257] = b2.astype(np.float32)
    w2_h = w2_h.reshape(KC, 128, D)

    # WG[f, h, o] = sum_d wv_pad[f, h*256+d] * fw[o, h*256+d]
    wv_pad = np.zeros((258, HD), np.float64)
    wv_pad[0:257] = wv_flat.T
    wv_pad[257] = wv_b
    wg_h = np.einsum('fhd,ohd->fho', wv_pad.reshape(258, H, D),
                     fw.reshape(D, H, D)).astype(np.float32)
    wg_full = np.zeros((KC * 128, H, D), np.float32)
    wg_full[0:258] = wg_h
    wg_full = wg_full.reshape(KC, 128, H, D)

    zt2 = np.zeros((128, 16, 8), np.float32)
    ind = np.zeros((128, 8, 8), np.float32)
    for hh in range(8):
        ind[:, hh, hh] = 1.0
    ind2 = np.zeros((8, 8, 128), np.float32)
    for hh in range(8):
        ind2[hh, hh, :] = 1.0

    s = abs(float(np.asarray(norm_scale))) + EPS
    eps_eff = EPS * s * s
    cons = np.full((8, 1), eps_eff, np.float32)

    common = {
        "wq": wq_h, "wk": wk_h, "w2": w2_h, "wg": wg_full,
        "zt2": zt2, "ind": ind, "ind2": ind2,
        "ident": np.eye(128, dtype=np.float32),
        "cons": cons,
    }
    in_maps = []
    for c in range(NCORES):
        m = dict(common)
        m["xqT"] = np.ascontiguousarray(xqT[:, :, c * NCHUNK:(c + 1) * NCHUNK])
        m["xsT"] = np.ascontiguousarray(xsT[:, :, c * NCHUNK:(c + 1) * NCHUNK])
        m["xnm"] = np.ascontiguousarray(xnm[c * NCHUNK:(c + 1) * NCHUNK, :])
        in_maps.append(m)
    return in_maps


def kernel(reps=1, **inputs):
    nc = _build(reps)
    in_maps = _prep_inputs(**inputs)
    res = run_bass_kernel_spmd(nc, in_maps, list(range(NCORES)))
    return np.concatenate([res.results[c]["out"] for c in range(NCORES)], axis=0)


# ---------------------------------------------------------------------------
# Cached-executable runner (used by test.py for accurate HW timing).
# run_bass_kernel_spmd rebuilds its jit on every call, which re-lowers and
# re-uploads everything; for timing we keep one jitted executable per reps
# value and re-invoke it, so repeat calls measure device execution.

def make_cached_runner(reps, inputs, donate=False):
    import jax
    from jax.sharding import Mesh, PartitionSpec, NamedSharding
    from jax.experimental.shard_map import shard_map
    import concourse.bass2jax as b2j

    nc = _build(reps)
    in_maps = _prep_inputs(**inputs)
    b2j.install_neuronx_cc_hook()
    partition_name = nc.partition_id_tensor.name if nc.partition_id_tensor else None
    in_names, out_names, out_avals, zero_shapes = [], [], [], []
    for alloc in nc.m.functions[0].allocations:
        if not isinstance(alloc, mybir.MemoryLocationSet):
            continue
        name = alloc.memorylocations[0].name
        if alloc.kind == "ExternalInput":
            if name != partition_name:
                in_names.append(name)
        elif alloc.kind == "ExternalOutput":
            shape = tuple(alloc.tensor_shape)
            dtype = mybir.dt.np(alloc.dtype)
            out_names.append(name)
            out_avals.append(jax.core.ShapedArray(shape, dtype))
            zero_shapes.append((shape, dtype))
    n_params = len(in_names)
    n_outs = len(out_avals)
    bind_names = list(in_names) + list(out_names)
    if partition_name is not None:
        bind_names.append(partition_name)

    def _bass_body(*args):
        operands = list(args)
        if partition_name is not None:
            operands.append(b2j.partition_id_tensor())
        outs = b2j._bass_exec_p.bind(
            *operands,
            out_avals=tuple(out_avals),
            in_names=tuple(bind_names),
            out_names=tuple(out_names),
            lowering_input_output_aliases=(),
            sim_require_finite=True,
            sim_require_nnan=True,
            nc=nc,
        )
        return tuple(outs)

    devices = jax.devices()[:NCORES]
    mesh = Mesh(np.asarray(devices), ("core",))
    in_specs = (PartitionSpec("core"),) * (n_params + n_outs)
    out_specs = (PartitionSpec("core"),) * n_outs
    donate_idx = tuple(range(n_params, n_params + n_outs)) if donate else ()
    jf = jax.jit(
        shard_map(_bass_body, mesh=mesh, in_specs=in_specs, out_specs=out_specs,
                  check_rep=False),
        donate_argnums=donate_idx, keep_unused=True,
    )
    sharding = NamedSharding(mesh, PartitionSpec("core"))
    per_core = [[np.asarray(m[name]) for name in in_names] for m in in_maps]
    concat_in = [np.concatenate([per_core[c][i] for c in range(NCORES)], axis=0)
                 for i in range(n_params)]
    dev_in = [jax.device_put(a, sharding) for a in concat_in]
    zeros = [jax.device_put(np.zeros((NCORES * s[0], *s[1:]), d), sharding)
             for (s, d) in zero_shapes]
    jax.block_until_ready(dev_in)
    jax.block_until_ready(zeros)

    def run():
        return jf(*dev_in, *zeros)

    def result_np(outs):
        return np.asarray(outs[0]).reshape(NCORES, *zero_shapes[0][0])

    return run, result_np


# revision 41
# speedup vs baseline: 1.1071x; 1.1071x over previous
"""Trainium2 Bass kernel for hyperbolic linear-attention transformer layer.

Data-parallel over nodes (N=32768) across 8 NeuronCores.

Math (per rep):
  Phase A (source nodes, node-major tiles of 128):
    k = Wk x_pad  (PE, fp32r) ; z = relu(k) (ACT)
    y = z^2 with per-head sums sy (DVE tensor_tensor_reduce)
    sy2 = sum(y^2) per head (ACT Square accum / DVE ttr)
    phi_k = y * sqrt(sy/sy2)   (in-place)
    B += x_pad^T phi_k         (PE, PSUM accumulation over node tiles)
      - x_pad has a trailing 1-column, so row 257 of B is sum_n(phi_k) "sumk"
  AllReduce(B) over the 8 cores  (2.1 MB, Shared output).
  Mid: G[h] = B_h^T WG_h where WG_h = Wv_pad_h fw_h^T (host-precomputed);
    this folds ktv = B^T Wv and the final projection fw into one matrix.
    sumk columns are DMA-staged into the stats lhsT (zt2).
  Phase B (query nodes, feature-major supertiles of 512):
    q = Wq x_pad (PE) ; z = relu(q) ; y = z^2 ; y2 = y^2
    A,C sums via one matmul per 128-feat chunk (lhsT = [ind | sumk]),
    Bsum via matmul vs y2.  fac = sqrt(A/Bsum); s = fac/(C*fac + eps).
    phi' = y * s (broadcast via K=8 matmul)
    outT = sum_h G_h^T phi'_h + W2 x_s  (W2 = fw @ (vmap Wv) folded on host,
      bias row included)
    PE-transpose outT -> node-major, Lorentz lift, DMA out.

All matmuls are fp32r with moving dim >= 256 (full PE rate).
"""

import os
import numpy as np
import concourse.bass as bass
import concourse.tile as tile
from concourse import bacc, mybir
from concourse.bass_utils import run_bass_kernel_spmd

F32 = mybir.dt.float32
F32R = mybir.dt.float32r
BF16 = mybir.dt.bfloat16
AF = mybir.ActivationFunctionType
ALU = mybir.AluOpType

NCORES = 8
N = 32768
NCHUNK = N // NCORES          # 4096 nodes per core
H = 8
D = 256
HD = H * D                    # 2048
KC = 3                        # contraction chunks: 384 = 3*128 (258 used)
EPS = 1e-6
NST = 512                     # phase-B supertile node count

_CACHE = {}


def _build(reps=1):
    if reps in _CACHE:
        return _CACHE[reps]
    nc = bacc.Bacc("TRN2", target_bir_lowering=False, debug=False,
                   num_devices=NCORES)

    xqT = nc.dram_tensor("xqT", [NCHUNK // 128, 128, KC, 128], F32R, kind="ExternalInput").ap()
    xsT = nc.dram_tensor("xsT", [NCHUNK // 128, 128, KC, 128], F32R, kind="ExternalInput").ap()
    xnm = nc.dram_tensor("xnm", [NCHUNK, KC * 128], F32R, kind="ExternalInput").ap()
    wq = nc.dram_tensor("wq", [KC, 128, HD], F32R, kind="ExternalInput").ap()
    wk = nc.dram_tensor("wk", [KC, 128, HD], F32R, kind="ExternalInput").ap()
    w2 = nc.dram_tensor("w2", [KC, 128, D], F32R, kind="ExternalInput").ap()
    wg = nc.dram_tensor("wg", [KC, 128, H, D], F32R, kind="ExternalInput").ap()
    zt2 = nc.dram_tensor("zt2", [128, 16, 8], F32R, kind="ExternalInput").ap()
    ind = nc.dram_tensor("ind", [128, 8, 8], F32R, kind="ExternalInput").ap()
    ind2 = nc.dram_tensor("ind2", [8, 8, 128], F32R, kind="ExternalInput").ap()
    ident = nc.dram_tensor("ident", [128, 128], F32R, kind="ExternalInput").ap()
    cons = nc.dram_tensor("cons", [8, 1], F32, kind="ExternalInput").ap()
    out = nc.dram_tensor("out", [NCHUNK, 257], F32, kind="ExternalOutput").ap()

    with tile.TileContext(nc) as tc:
        _body(nc, tc, reps, xqT, xsT, xnm, wq, wk, w2, wg, zt2, ind, ind2,
              ident, cons, out)
    nc.compile()
    _CACHE[reps] = nc
    return nc


def _body(nc, tc, reps, xqT, xsT, xnm, wq, wk, w2, wg, zt2, ind, ind2,
          ident, cons, out):
    import contextlib
    stack = contextlib.ExitStack()
    with stack:
        cpool = stack.enter_context(tc.tile_pool(name="const", bufs=1))
        dpool = stack.enter_context(tc.tile_pool(name="dram", bufs=1, space="DRAM"))

        ind_sb = cpool.tile([128, 8, 8], F32R)
        nc.sync.dma_start(ind_sb[:], ind[:])
        ind2_sb = cpool.tile([8, 8, 128], F32R)
        nc.sync.dma_start(ind2_sb[:], ind2[:])
        ident_sb = cpool.tile([128, 128], F32R)
        nc.sync.dma_start(ident_sb[:], ident[:])
        eps_sb = cpool.tile([8, 1], F32)
        nc.sync.dma_start(eps_sb[:], cons[:])
        zt_sb = cpool.tile([128, 16, 8], F32R)
        nc.sync.dma_start(zt_sb[:], zt2[:])

        for rep in range(reps):
            ar_in = [dpool.tile([258, 1024], BF16, tag=f"ari{rep}g{g}",
                                name=f"ari{rep}g{g}") for g in range(2)]
            ar_out = [dpool.tile([258, 1024], BF16,
                                 tag=f"aro{rep}g{g}", name=f"aro{rep}g{g}")
                      for g in range(2)]
            if not os.environ.get("KT_SKIP_A"):
                _phase_a(nc, tc, xsT, xnm, wk, ar_in, ar_out)
            if not os.environ.get("KT_SKIP_B"):
                _phase_b(nc, tc, xqT, xsT, wq, w2, wg, zt_sb, ind_sb, ind2_sb,
                         ident_sb, eps_sb, ar_out, out)
            else:
                with tc.tile_pool(name="oBtmp", bufs=1) as ob:
                    o_sb = ob.tile([128, 257], F32)
                    nc.sync.dma_start(o_sb[:], ar_out[0][0:128, 0:257])
                    for t0_ in range(NCHUNK // 128):
                        nc.sync.dma_start(out[t0_ * 128:(t0_ + 1) * 128, :], o_sb[:])


def _phase_a(nc, tc, xsT, xnm, wk, ar_in, ar_out):
    import contextlib
    with contextlib.ExitStack() as st:
        wpool = st.enter_context(tc.tile_pool(name="wA", bufs=1))
        xp = st.enter_context(tc.tile_pool(name="xA", bufs=3))
        xnp_ = st.enter_context(tc.tile_pool(name="xnA", bufs=4))
        yp = st.enter_context(tc.tile_pool(name="yA", bufs=3))
        zap = st.enter_context(tc.tile_pool(name="zA", bufs=2))
        scp = st.enter_context(tc.tile_pool(name="scA", bufs=2))
        stp = st.enter_context(tc.tile_pool(name="stA", bufs=4))
        drp = st.enter_context(tc.tile_pool(name="drA", bufs=2))
        pk = st.enter_context(tc.tile_pool(name="psAk", bufs=2, space="PSUM"))
        pb = st.enter_context(tc.tile_pool(name="psAb", bufs=1, space="PSUM"))

        wk_sb = wpool.tile([128, KC, HD], F32R)
        nc.sync.dma_start(wk_sb[:], wk.rearrange("c p n -> p c n"))

        ntiles = NCHUNK // 128
        for g in range(2):
            gofs = g * 1024
            b_ps0 = pb.tile([128, 1024], F32, tag="bps0")
            b_ps1 = pb.tile([128, 1024], F32, tag="bps1")
            b_ps2 = pb.tile([2, 1024], F32, tag="bps2")
            b_tiles = (b_ps0, b_ps1, b_ps2)
            prev = None
            onex = bool(os.environ.get("KT_ONEX"))
            for t in range(ntiles):
                tsrc = 0 if onex else t
                xs_sb = xp.tile([128, KC, 128], F32R, tag="xs")
                nc.sync.dma_start(xs_sb[:], xsT[tsrc])
                xn_sb = xnp_.tile([128, KC, 128], F32R, tag="xn")
                nc.sync.dma_start(
                    xn_sb[:],
                    xnm[tsrc * 128:(tsrc + 1) * 128, :].rearrange(
                        "n (c f) -> n c f", c=KC))

                y = yp.tile([128, 1024], F32R, tag="y")
                yf = y.bitcast(F32)
                z = zap.tile([128, 1024], F32, tag="zA")
                sy = stp.tile([128, 4], F32, tag="sy")
                sy2 = stp.tile([128, 4], F32, tag="sy2")
                for blk in range(2):
                    kp = pk.tile([128, 512], F32, tag="kp")
                    for c in range(KC):
                        nc.tensor.matmul(
                            kp[:], lhsT=xs_sb[:, c],
                            rhs=wk_sb[:, c, gofs + blk * 512: gofs + blk * 512 + 512],
                            start=(c == 0), stop=(c == KC - 1))
                    nc.vector.tensor_scalar_max(
                        z[:, blk * 512:(blk + 1) * 512], kp[:], 0.0)
                if os.environ.get("KT_NOSTATS"):
                    for hh in range(4):
                        sl = slice(hh * 256, hh * 256 + 256)
                        nc.scalar.activation(y[:, sl], z[:, sl], AF.Square)
                else:
                    # y = z^2 with per-head accumulated sums sy (ACT)
                    for hh in range(4):
                        sl = slice(hh * 256, hh * 256 + 256)
                        nc.scalar.activation(y[:, sl], z[:, sl], AF.Square,
                                             accum_out=sy[:, hh:hh + 1])
                    # sy2 = sum(y^2) per head (ACT Square with accum)
                    for hh in range(4):
                        sl = slice(hh * 256, hh * 256 + 256)
                        scr = scp.tile([128, 256], F32, tag="scr")
                        nc.scalar.activation(scr[:], yf[:, sl], AF.Square,
                                             accum_out=sy2[:, hh:hh + 1])
                    rec = stp.tile([128, 4], F32, tag="rec")
                    nc.vector.reciprocal(rec[:], sy2[:])
                    rat = stp.tile([128, 4], F32, tag="rat")
                    nc.vector.tensor_mul(rat[:], sy[:], rec[:])
                    fac = stp.tile([128, 4], F32, tag="fac")
                    nc.scalar.activation(fac[:], rat[:], AF.Sqrt)
                    # phi = y * fac, in place (2 on DVE, 2 on ACT)
                    for hh in range(4):
                        sl = slice(hh * 256, hh * 256 + 256)
                        if hh < 2:
                            nc.vector.tensor_scalar_mul(y[:, sl], yf[:, sl],
                                                        fac[:, hh:hh + 1])
                        else:
                            nc.scalar.activation(y[:, sl], yf[:, sl], AF.Copy,
                                                 scale=fac[:, hh:hh + 1])

                # B accumulation for the PREVIOUS tile (software pipeline:
                # keeps PE busy with tile t's projection while tile t-1's
                # phi chain drains on DVE/ACT)
                if os.environ.get("KT_NO_B"):
                    if t == 0:
                        _b_accum(nc, b_tiles, xn_sb, y, 0, 1)
                elif t > 0:
                    _b_accum(nc, b_tiles, prev[0], prev[1], t - 1, ntiles)
                prev = (xn_sb, y)
            if not os.environ.get("KT_NO_B"):
                _b_accum(nc, b_tiles, prev[0], prev[1], ntiles - 1, ntiles)

            d0 = drp.tile([128, 1024], BF16, tag="d0")
            nc.vector.tensor_copy(d0[:], b_ps0[:])
            nc.sync.dma_start(ar_in[g][0:128, :], d0[:])
            d1 = drp.tile([128, 1024], BF16, tag="d1")
            nc.scalar.copy(d1[:], b_ps1[:])
            nc.sync.dma_start(ar_in[g][128:256, :], d1[:])
            d2 = drp.tile([2, 1024], BF16, tag="d2")
            nc.vector.tensor_copy(d2[:], b_ps2[:])
            nc.sync.dma_start(ar_in[g][256:258, :], d2[:])
            # per-group AllReduce: group 0's AR overlaps group 1's compute
            if os.environ.get("KT_LOCAL_AR"):
                nc.sync.dma_start(ar_out[g][:], ar_in[g][:])
            else:
                nc.gpsimd.collective_compute(
                    "AllReduce", ALU.add,
                    replica_groups=[list(range(NCORES))],
                    ins=[ar_in[g].opt()], outs=[ar_out[g].opt()])


def _b_accum(nc, b_tiles, xn_sb, y, t, ntiles):
    b_ps0, b_ps1, b_ps2 = b_tiles
    for blk in range(2):
        ms = slice(blk * 512, blk * 512 + 512)
        nc.tensor.matmul(b_ps0[:, ms], lhsT=xn_sb[:, 0], rhs=y[:, ms],
                         start=(t == 0), stop=(t == ntiles - 1))
        nc.tensor.matmul(b_ps1[:, ms], lhsT=xn_sb[:, 1], rhs=y[:, ms],
                         start=(t == 0), stop=(t == ntiles - 1))
        nc.tensor.matmul(b_ps2[:, ms], lhsT=xn_sb[:, 2, 0:2], rhs=y[:, ms],
                         start=(t == 0), stop=(t == ntiles - 1))


def _phase_b(nc, tc, xqT, xsT, wq, w2, wg, zt_sb, ind_sb, ind2_sb,
             ident_sb, eps_sb, ar_out, out):
    import contextlib
    with contextlib.ExitStack() as st:
        wpool = st.enter_context(tc.tile_pool(name="wB", bufs=1))
        mpool = st.enter_context(tc.tile_pool(name="midB", bufs=1))
        xp = st.enter_context(tc.tile_pool(name="xB", bufs=2))
        yp = st.enter_context(tc.tile_pool(name="yB", bufs=34))
        zbp = st.enter_context(tc.tile_pool(name="zbB", bufs=3))
        scp2 = st.enter_context(tc.tile_pool(name="sc2B", bufs=2))
        stp = st.enter_context(tc.tile_pool(name="stB", bufs=1))
        sbp = st.enter_context(tc.tile_pool(name="sbB", bufs=2))
        asb = st.enter_context(tc.tile_pool(name="aB", bufs=3))
        obp = st.enter_context(tc.tile_pool(name="oB", bufs=3))
        ps = st.enter_context(tc.tile_pool(name="psBs", bufs=2, space="PSUM"))
        pa = st.enter_context(tc.tile_pool(name="psBa", bufs=3, space="PSUM"))
        pst = st.enter_context(tc.tile_pool(name="psBt", bufs=2, space="PSUM"))

        wq_sb = wpool.tile([128, KC, HD], F32R)
        nc.sync.dma_start(wq_sb[:], wq.rearrange("c p n -> p c n"))
        w2_sb = wpool.tile([128, KC, D], F32R)
        nc.sync.dma_start(w2_sb[:], w2.rearrange("c p n -> p c n"))
        g_sb = wpool.tile([128, H, 2, D], F32R)

        def mid_g(g):
            # G = B^T WG for this half's heads; stage sumk cols into zt
            bf0 = mpool.tile([128, 1024], F32R, tag="bf0")
            nc.gpsimd.dma_start(bf0[:], ar_out[g][0:128, :])
            bf1 = mpool.tile([128, 1024], F32R, tag="bf1")
            nc.gpsimd.dma_start(bf1[:], ar_out[g][128:256, :])
            bf2 = mpool.tile([2, 1024], F32R, tag="bf2")
            nc.gpsimd.dma_start(bf2[:], ar_out[g][256:258, :])
            wg_sb = mpool.tile([128, KC, 4, D], F32R, tag="wgh")
            nc.sync.dma_start(
                wg_sb[:], wg[:, :, g * 4:(g + 1) * 4].rearrange(
                    "c p h n -> p c h n"))
            bfs = [bf0, bf1, bf2]
            for hl in range(4):
                hh = g * 4 + hl
                for mc in range(2):
                    msl = slice(hl * 256 + mc * 128, hl * 256 + mc * 128 + 128)
                    gp = pa.tile([128, NST], F32, tag="mm")
                    for fc in range(KC):
                        lhs = bfs[fc][:, msl] if fc < 2 else bfs[2][0:2, msl]
                        rhs = (wg_sb[:, fc, hl] if fc < 2
                               else wg_sb[0:2, fc, hl])
                        nc.tensor.matmul(gp[:, 0:D], lhsT=lhs, rhs=rhs,
                                         start=(fc == 0), stop=(fc == KC - 1))
                    if (hl * 2 + mc) % 2 == 0:
                        nc.vector.tensor_copy(g_sb[:, hh, mc], gp[:, 0:D])
                    else:
                        nc.scalar.copy(g_sb[:, hh, mc], gp[:, 0:D])
            for cl in range(8):
                c = g * 8 + cl
                hh = c // 2
                nc.gpsimd.dma_start(
                    zt_sb[:, c, hh:hh + 1],
                    ar_out[g][257:258, cl * 128:(cl + 1) * 128].rearrange(
                        "r (p o) -> (r p) o", o=1))

        state = {}

        def pre(stx):
            nofs = stx * NST
            xq_sb = xp.tile([128, KC, 4, 128], F32R, tag="xq")
            nc.sync.dma_start(
                xq_sb[:],
                xqT[stx * 4:(stx + 1) * 4].rearrange("t p c n -> p c t n"))
            ys = []
            for c in range(16):
                qp = pa.tile([128, NST], F32, tag="mm")
                for kc in range(KC):
                    nc.tensor.matmul(
                        qp[:], lhsT=wq_sb[:, kc, c * 128:(c + 1) * 128],
                        rhs=xq_sb[:, kc], start=(kc == 0), stop=(kc == KC - 1))
                zb = zbp.tile([128, NST], F32, tag="zbB")
                if c % 4 == 3:
                    nc.scalar.activation(zb[:], qp[:], AF.Relu)
                else:
                    nc.vector.tensor_scalar_max(zb[:], qp[:], 0.0)
                y = yp.tile([128, NST], F32R, tag="yB")
                nc.scalar.activation(y[:], zb[:], AF.Square)
                ys.append(y)
            state[stx] = ys

        def post(stx):
            nofs = stx * NST
            ys = state.pop(stx)
            # C = sumk-weighted sums of y; query-side fp-normalization cancels
            # between numerator and denominator, so s = 1/(C + eps) directly
            sums_ps = ps.tile([8, NST], F32, tag="sums")
            for c in range(16):
                nc.tensor.matmul(sums_ps[:], lhsT=zt_sb[:, c], rhs=ys[c][:],
                                 start=(c == 0), stop=(c == 15))
            den = stp.tile([8, NST], F32, tag="denB")
            nc.vector.tensor_scalar_add(den[:], sums_ps[:], eps_sb[:])
            s_sb = stp.tile([8, NST], F32R, tag="sB")
            with nc.allow_low_precision(reason="f32r rounding for matmul rhs"):
                nc.vector.reciprocal(s_sb[:], den[:])

            # phi' = y * s (broadcast s across partitions via K=8 matmul)
            for hh in range(8):
                sb_ps = pst.tile([128, NST], F32, tag="sbtr")
                nc.tensor.matmul(sb_ps[:], lhsT=ind2_sb[:, hh], rhs=s_sb[:],
                                 start=True, stop=True)
                sbc = sbp.tile([128, NST], F32, tag="sbcs")
                if hh % 4 == 3:
                    nc.vector.tensor_copy(sbc[:], sb_ps[:])
                else:
                    nc.scalar.copy(sbc[:], sb_ps[:])
                for mc in range(2):
                    yo = ys[2 * hh + mc]
                    nc.vector.tensor_mul(yo[:], yo.bitcast(F32)[:], sbc[:])

            # outT accumulation: vss (W2 x_s) then numerator via G
            xs_sb = xp.tile([128, KC, 4, 128], F32R, tag="xsB")
            nc.sync.dma_start(
                xs_sb[:],
                xsT[stx * 4:(stx + 1) * 4].rearrange("t p c n -> p c t n"))
            at_sbs = []
            for oc in range(2):
                osl = slice(oc * 128, oc * 128 + 128)
                at_ps = pa.tile([128, NST], F32, tag="mm")
                for fc in range(KC):
                    nc.tensor.matmul(at_ps[:], lhsT=w2_sb[:, fc, osl],
                                     rhs=xs_sb[:, fc],
                                     start=(fc == 0), stop=False)
                for hh in range(8):
                    for mc in range(2):
                        nc.tensor.matmul(
                            at_ps[:], lhsT=g_sb[:, hh, mc, osl],
                            rhs=ys[2 * hh + mc][:],
                            start=False, stop=(hh == 7 and mc == 1))
                at_sb = asb.tile([128, NST], F32R, tag="atB")
                if oc == 0:
                    nc.scalar.copy(at_sb[:], at_ps[:])
                else:
                    nc.vector.tensor_copy(at_sb[:], at_ps[:])
                at_sbs.append(at_sb)

            # transpose to node-major + Lorentz lift
            for sn in range(NST // 128):
                tr_ps = pst.tile([128, 2, 128], F32R, tag="sbtr")
                for oc in range(2):
                    nc.tensor.transpose(
                        tr_ps[:, oc],
                        at_sbs[oc][:, sn * 128:(sn + 1) * 128], ident_sb[:])
                trf = tr_ps.bitcast(F32).rearrange("p a b -> p (a b)")
                o_sb = obp.tile([128, 257], F32, tag="osb")
                nc.vector.tensor_copy(o_sb[:, 1:257], trf)
                scr2 = scp2.tile([128, 256], F32, tag="scr2")
                ssum = stp.tile([128, 1], F32, tag="ssum")
                nc.scalar.activation(scr2[:], trf, AF.Square, accum_out=ssum[:])
                nc.scalar.activation(o_sb[:, 0:1], ssum[:], AF.Sqrt, bias=1.0)
                nc.sync.dma_start(out[nofs + sn * 128: nofs + (sn + 1) * 128, :],
                                  o_sb[:])

        # software pipeline: pres are AR-independent, posts need G/zt (AR)
        nst = NCHUNK // NST
        mid_g(0)
        pre(0)
        pre(1)
        mid_g(1)
        post(0)
        for stx in range(2, nst):
            pre(stx)
            post(stx - 1)
        post(nst - 1)


def _prep_inputs(query_input, source_input, Wq_w, Wq_b, Wk_w, Wk_b, Wv_w, Wv_b,
                 norm_scale, v_map_w, v_map_b, final_w, final_b):
    def pad_xT(x):
        xt = np.zeros((KC * 128, N), np.float32)
        xt[0:257] = x.T
        xt[257] = 1.0
        return xt.reshape(KC, 128, N)

    def pad_w(w_flat, b_flat):
        wt = np.zeros((KC * 128, HD), np.float32)
        wt[0:257] = w_flat.T
        wt[257] = b_flat
        return wt.reshape(KC, 128, HD)

    xq = np.asarray(query_input, np.float32)
    xs = np.asarray(source_input, np.float32)
    xqT = np.ascontiguousarray(
        pad_xT(xq).reshape(KC, 128, N // 128, 128).transpose(2, 1, 0, 3))
    xsT = np.ascontiguousarray(
        pad_xT(xs).reshape(KC, 128, N // 128, 128).transpose(2, 1, 0, 3))
    xnm = np.zeros((N, KC * 128), np.float32)
    xnm[:, 0:257] = xs
    xnm[:, 257] = 1.0

    wq_h = pad_w(np.asarray(Wq_w).reshape(HD, 257), np.asarray(Wq_b).reshape(HD))
    wk_h = pad_w(np.asarray(Wk_w).reshape(HD, 257), np.asarray(Wk_b).reshape(HD))

    fw = np.asarray(final_w, np.float64)                     # [256, 2048]
    vm = np.asarray(v_map_w, np.float64)
    wv_flat = np.asarray(Wv_w, np.float64).reshape(HD, 257)  # [2048, 257]
    wv_b = np.asarray(Wv_b, np.float64).reshape(HD)

    # W2 = fw @ (vm @ Wv) : [256, 257]; bias = fw @ (vm @ Wv_b + v_map_b) + final_b
    wvm_flat = np.einsum('od,hdi->hoi', vm, np.asarray(Wv_w, np.float64)
                         ).reshape(HD, 257)
    bvm = (np.asarray(Wv_b, np.float64) @ vm.T
           + np.asarray(v_map_b, np.float64)[None, :]).reshape(HD)
    w2m = fw @ wvm_flat                                      # [256, 257]
    b2 = fw @ bvm + np.asarray(final_b, np.float64)          # [256]
    w2_h = np.zeros((KC * 128, D), np.float32)
    w2_h[0:257] = w2m.T.astype(np.float32)
    w2_h[257] = b2.astype(np.float32)
    w2_h = w2_h.reshape(KC, 128, D)

    # WG[f, h, o] = sum_d wv_pad[f, h*256+d] * fw[o, h*256+d]
    wv_pad = np.zeros((258, HD), np.float64)
    wv_pad[0:257] = wv_flat.T
    wv_pad[257] = wv_b
    wg_h = np.einsum('fhd,ohd->fho', wv_pad.reshape(258, H, D),
                     fw.reshape(D, H, D)).astype(np.float32)
    wg_full = np.zeros((KC * 128, H, D), np.float32)
    wg_full[0:258] = wg_h
    wg_full = wg_full.reshape(KC, 128, H, D)

    zt2 = np.zeros((128, 16, 8), np.float32)
    ind = np.zeros((128, 8, 8), np.float32)
    for hh in range(8):
        ind[:, hh, hh] = 1.0
    ind2 = np.zeros((8, 8, 128), np.float32)
    for hh in range(8):
        ind2[hh, hh, :] = 1.0

    s = abs(float(np.asarray(norm_scale))) + EPS
    eps_eff = EPS * s * s
    cons = np.full((8, 1), eps_eff, np.float32)

    common = {
        "wq": wq_h, "wk": wk_h, "w2": w2_h, "wg": wg_full,
        "zt2": zt2, "ind": ind, "ind2": ind2,
        "ident": np.eye(128, dtype=np.float32),
        "cons": cons,
    }
    in_maps = []
    for c in range(NCORES):
        m = dict(common)
        ntc = NCHUNK // 128
        m["xqT"] = np.ascontiguousarray(xqT[c * ntc:(c + 1) * ntc])
        m["xsT"] = np.ascontiguousarray(xsT[c * ntc:(c + 1) * ntc])
        m["xnm"] = np.ascontiguousarray(xnm[c * NCHUNK:(c + 1) * NCHUNK, :])
        in_maps.append(m)
    return in_maps


def kernel(reps=1, **inputs):
    nc = _build(reps)
    in_maps = _prep_inputs(**inputs)
    res = run_bass_kernel_spmd(nc, in_maps, list(range(NCORES)))
    return np.concatenate([res.results[c]["out"] for c in range(NCORES)], axis=0)


# ---------------------------------------------------------------------------
# Cached-executable runner (used by test.py for accurate HW timing).
# run_bass_kernel_spmd rebuilds its jit on every call, which re-lowers and
# re-uploads everything; for timing we keep one jitted executable per reps
# value and re-invoke it, so repeat calls measure device execution.

def make_cached_runner(reps, inputs, donate=False):
    import jax
    from jax.sharding import Mesh, PartitionSpec, NamedSharding
    from jax.experimental.shard_map import shard_map
    import concourse.bass2jax as b2j

    nc = _build(reps)
    in_maps = _prep_inputs(**inputs)
    b2j.install_neuronx_cc_hook()
    partition_name = nc.partition_id_tensor.name if nc.partition_id_tensor else None
    in_names, out_names, out_avals, zero_shapes = [], [], [], []
    for alloc in nc.m.functions[0].allocations:
        if not isinstance(alloc, mybir.MemoryLocationSet):
            continue
        name = alloc.memorylocations[0].name
        if alloc.kind == "ExternalInput":
            if name != partition_name:
                in_names.append(name)
        elif alloc.kind == "ExternalOutput":
            shape = tuple(alloc.tensor_shape)
            dtype = mybir.dt.np(alloc.dtype)
            out_names.append(name)
            out_avals.append(jax.core.ShapedArray(shape, dtype))
            zero_shapes.append((shape, dtype))
    n_params = len(in_names)
    n_outs = len(out_avals)
    bind_names = list(in_names) + list(out_names)
    if partition_name is not None:
        bind_names.append(partition_name)

    def _bass_body(*args):
        operands = list(args)
        if partition_name is not None:
            operands.append(b2j.partition_id_tensor())
        outs = b2j._bass_exec_p.bind(
            *operands,
            out_avals=tuple(out_avals),
            in_names=tuple(bind_names),
            out_names=tuple(out_names),
            lowering_input_output_aliases=(),
            sim_require_finite=True,
            sim_require_nnan=True,
            nc=nc,
        )
        return tuple(outs)

    devices = jax.devices()[:NCORES]
    mesh = Mesh(np.asarray(devices), ("core",))
    in_specs = (PartitionSpec("core"),) * (n_params + n_outs)
    out_specs = (PartitionSpec("core"),) * n_outs
    donate_idx = tuple(range(n_params, n_params + n_outs)) if donate else ()
    jf = jax.jit(
        shard_map(_bass_body, mesh=mesh, in_specs=in_specs, out_specs=out_specs,
                  check_rep=False),
        donate_argnums=donate_idx, keep_unused=True,
    )
    sharding = NamedSharding(mesh, PartitionSpec("core"))
    per_core = [[np.asarray(m[name]) for name in in_names] for m in in_maps]
    concat_in = [np.concatenate([per_core[c][i] for c in range(NCORES)], axis=0)
                 for i in range(n_params)]
    dev_in = [jax.device_put(a, sharding) for a in concat_in]
    zeros = [jax.device_put(np.zeros((NCORES * s[0], *s[1:]), d), sharding)
             for (s, d) in zero_shapes]
    jax.block_until_ready(dev_in)
    jax.block_until_ready(zeros)

    def run():
        return jf(*dev_in, *zeros)

    def result_np(outs):
        return np.asarray(outs[0]).reshape(NCORES, *zero_shapes[0][0])

    return run, result_np
